# revision 14
# baseline (speedup 1.0000x reference)
"""MoE expert FFN (swiglu) kernel for 8 trn2 NeuronCores.

Expert parallelism: 8 experts, one per core. Each core computes, for its
expert e:
    h   = x_e @ w1_e            # [2048, 2048] @ [2048, 2816]
    act = silu(h[:, :1408]) * h[:, 1408:]
    out = act @ w2_e            # [2048, 1408] @ [1408, 2048]

Tokens arrive pre-sorted by expert with equal counts (2048/expert), so
sharding is a static slice and the gather is a concat. No collectives.

Device-side layout (all bf16 compute, fp32 PSUM accumulation, fp32 out):
  mm1: out[f, t] tiles; lhsT = w1[h,f] 128x128 tiles (stationary),
       rhs = xT[h, t] (moving, N=512) -> inter is [f, t], the layout mm2
       needs, so no on-device transpose anywhere (x is transposed on host).
  swiglu pairs: w1 columns are interleaved on HOST so pair j = cols
       [256j, 256j+256) = [a_j | b_j]; act_j = silu(a)*b via ACT(Silu)
       + DVE mul -> bf16 SBUF.
  mm2: out[t, h] tiles; lhsT = act[f, t] 128-col slices (stationary),
       rhs = w2[f, h] (moving, N=512). PSUM -> SBUF f32 -> DMA to out.

v5 scheduling. Measured DMA aggregate is only ~220-300GB/s shared by all
queues, so a 512-token first chunk consumes w1 (one 3-pair block per
20.4us) faster than it can stream -> 15-25us of PE stalls (v3/v4). Fix:
process token chunks JOINTLY in super-chunks of 1024 tokens (chunks 0+1,
then 2+3). mm1 advances one swiglu pair per FOUR psum chains (a0,b0,a1,
b1 over both 512-chunks), halving the required w1 block cadence to
40.9us, which the DMA sustains with slack:
  - sync  HWDGE: b0 k-even interleaved with x1 k-even, then b1/b2
    k-even, w2 k-even, then x2+x3 (super-chunk 1, needed ~100us later).
  - scalar HWDGE: same with k-odd, then output stores.
  - gpsimd SWDGE: x0, then w1 b3 (needed only ~150us in), then spare.
  PSUM: quad = 4 banks + 4 draining = 8. act/x SBUF slots are reused
  across super-chunks (bufs=1; Tile inserts the release waits).
  mm2 runs per super-chunk (8 m-tiles); the very last m-tile is n-outer
  so its PSUM banks drain while the PE finishes -> shorter tail.

Weights stay resident in SBUF (bf16: 88KB + 44KB per partition).
PE-bound: ~451us of matmul per core at 2.4GHz; target is wall ~= that.
"""

import os
import sys

sys.path.insert(0, "/opt/trn_rl_repo")

import numpy as np
import ml_dtypes

E = 8             # experts == cores
T_TOTAL = 16384
H = 2048
F = 1408
F2 = 2 * F        # 2816
TPC = T_TOTAL // E  # 2048 tokens per core
CHUNK = 512
NSC = 2                     # super-chunks
NCI = 2                     # chunks per super-chunk
KH = H // 128               # 16 contraction tiles for mm1
NF = F // 128               # 11 swiglu pairs
NT = (NCI * CHUNK) // 128   # 8 m-tiles per super-chunk in mm2
NHO = H // 512              # 4 output column blocks

# w1 column blocks (in interleaved-pair space). The first three pairs get
# their own 256-col tile so quad j only waits for pair j's 16 k-slices
# during the DMA-paced startup; later pairs use 768/512-col blocks (fewer
# DMAs, arrive with slack).
W1_BLOCKS = [(0, 1), (1, 1), (2, 1), (3, 3), (6, 3), (9, 2)]  # (first pair, n)
W1_PAIR_BLOCK = [0, 1, 2, 3, 3, 3, 4, 4, 4, 5, 5]  # pair j -> block index

_CACHE = {}

# Optional knobs read by test.py (not used by the grading harness).
TRACE = os.environ.get("BASS_TRACE_KERNEL", "0") == "1"
LAST = {}


def _build():
    from concourse import bacc, tile, mybir

    bf16 = mybir.dt.bfloat16
    f32 = mybir.dt.float32
    SILU = mybir.ActivationFunctionType.Silu

    # Bacc (not plain Bass): its lowering pipeline splits multi-sem waits
    # into EventSemaphore pairs — TRN2 allows at most 1 wait per instruction.
    nc = bacc.Bacc()
    xT_d = nc.declare_dram_parameter("xT", [H, TPC], bf16, isOutput=False)
    w1_d = nc.declare_dram_parameter("w1", [H, F2], bf16, isOutput=False)
    w2_d = nc.declare_dram_parameter("w2", [F, H], bf16, isOutput=False)
    out_d = nc.declare_dram_parameter("out", [TPC, H], f32, isOutput=True)

    def x_dram(c, k):
        return xT_d[k * 128 : (k + 1) * 128, c * CHUNK : (c + 1) * CHUNK]

    with tile.TileContext(nc) as tc:
        with (
            tc.tile_pool(name="w1p", bufs=1) as w1p,
            tc.tile_pool(name="w2p", bufs=1) as w2p,
            tc.tile_pool(name="xp", bufs=1) as xp,
            tc.tile_pool(name="actp", bufs=1) as actp,
            tc.tile_pool(name="tmpp", bufs=2) as tmpp,
            tc.tile_pool(name="outp", bufs=4) as outp,
            tc.tile_pool(name="psp", bufs=8, space="PSUM") as psp,
        ):
            # x chunk 0 on gpsimd (SWDGE): both HWDGE queues stay free for
            # w1/x1, which gate the first swiglu quads.
            x0_t = []
            for k in range(KH):
                t = xp.tile([128, CHUNK], bf16, tag=f"x_0_{k}", name=f"x0_{k}")
                x0_t.append(t)
                nc.gpsimd.dma_start(out=t[:], in_=x_dram(0, k))

            # w1 blocks b0-b2 + x chunk 1, streamed in PE consumption order
            # on the two HWDGE queues (k-parity split). x1 k-slices are
            # interleaved into the b0 stream since quad j consumes
            # (w1_k, x0_k, x1_k) together. w1 b3 rides gpsimd after x0.
            #
            # CRITICAL scheduling constraint (cost v5 15us of PE stall):
            # the scalar HWDGE configs and the swiglu ACTIVATEs share ONE
            # sequential instruction stream, and each config beyond the
            # 4-slot rotation WAITS for an older transfer. Any scalar DMA
            # emitted before the mm1 loop therefore delays the first silu
            # (and with it the PSUM drain the PE depends on) by tens of
            # us. So scalar gets ONLY the b0-odd/x1-odd front-load here;
            # its remaining transfers (b1/b2-odd, w2-odd) are emitted
            # inside the mm1 loop in batches of <=4 (the slot count) after
            # each quad's swiglu, where the waits are long satisfied.
            w1_t = [[None] * len(W1_BLOCKS) for _ in range(KH)]
            x1_t = []
            for k in range(KH):
                t = xp.tile([128, CHUNK], bf16, tag=f"x_1_{k}", name=f"x1_{k}")
                x1_t.append(t)
            w1_dma = {}  # (b, k) -> (tile, dram slice), deferred issue
            for b, (p0, npair) in enumerate(W1_BLOCKS):
                c0, cw = p0 * 256, npair * 256
                for k in range(KH):
                    t = w1p.tile([128, cw], bf16, tag=f"w1_{k}_{b}")
                    w1_t[k][b] = t
                    src = w1_d[k * 128 : (k + 1) * 128, c0 : c0 + cw]
                    if b == len(W1_BLOCKS) - 1:
                        nc.gpsimd.dma_start(out=t[:], in_=src)
                    elif k % 2 == 0:
                        nc.sync.dma_start(out=t[:], in_=src)
                    elif b <= 2:
                        nc.scalar.dma_start(out=t[:], in_=src)
                    else:
                        w1_dma[(b, k)] = (t, src)
            # x chunk 1 rides both HWDGE queues AFTER the first three w1
            # pairs: the stagger below delays its first use to step 2
            # (~34us), so it must not compete with p0-p2 in the front-load.
            for k in range(KH):
                eng = nc.sync if k % 2 == 0 else nc.scalar
                eng.dma_start(out=x1_t[k][:], in_=x_dram(1, k))
            # Resident w2: 11 tiles [128, 2048]; even k on sync after w1,
            # odd k deferred into the mm1 loop (scalar) — not needed until
            # mm2 of super-chunk 0 (~190us in).
            w2_t = []
            w2_dma = []
            for k in range(NF):
                t = w2p.tile([128, H], bf16, tag=f"w2_{k}")
                w2_t.append(t)
                src = w2_d[k * 128 : (k + 1) * 128, :]
                if k % 2 == 0:
                    nc.sync.dma_start(out=t[:], in_=src)
                else:
                    w2_dma.append((t, src))
            # Scalar-queue batches (<=4 configs each) emitted after step
            # t's swiglu in super-chunk 0: 768-block odds, then w2-odd.
            deferred = [w1_dma[(b, k)] for b in (3, 4) for k in range(1, KH, 2)]
            deferred += w2_dma
            scalar_batches = [deferred[i : i + 4] for i in range(0, len(deferred), 4)]

            for S in range(NSC):
                if S == 0:
                    x_t = [x0_t, x1_t]
                else:
                    # Chunks 2+3 reuse chunk 0+1's SBUF slots (released at
                    # the end of super-chunk 0's mm1); the SP queue is idle
                    # from ~90us so the waits cost nothing.
                    x_t = [[], []]
                    for i in range(NCI):
                        for k in range(KH):
                            t = xp.tile(
                                [128, CHUNK], bf16, tag=f"x_{i}_{k}",
                                name=f"x_{S}_{i}_{k}",
                            )
                            x_t[i].append(t)
                            nc.sync.dma_start(out=t[:], in_=x_dram(S * NCI + i, k))

                # mm1 + swiglu, software-staggered: step t runs pair t of
                # chunk 0 and pair t-2 of chunk 1. Chunk 1's weights are
                # always two pairs old (resident), so only (x0, p0..p2)
                # are on the DMA critical path at startup — x1 isn't
                # needed until step 2 (~34us). Weight demand per block
                # still averages half the v3 rate, which the shared
                # ~290GB/s DMA sustains without PE stalls.
                STAG = 2
                act_t = [[None] * NF, [None] * NF]  # [chunk][j]
                for t_s in range(NF + STAG):
                    chains = []
                    if t_s < NF:
                        chains.append((0, t_s))
                    if t_s >= STAG:
                        chains.append((1, t_s - STAG))
                    for i, j in chains:
                        b = W1_PAIR_BLOCK[j]
                        off = (j - W1_BLOCKS[b][0]) * 256
                        ps_a = psp.tile(
                            [128, CHUNK], f32, tag="ps", name=f"ps_{S}_{i}_{j}_a"
                        )
                        ps_b = psp.tile(
                            [128, CHUNK], f32, tag="ps", name=f"ps_{S}_{i}_{j}_b"
                        )
                        for k in range(KH):
                            w1k = w1_t[k][b]
                            st, sp = (k == 0), (k == KH - 1)
                            xk = x_t[i][k][:]
                            nc.tensor.matmul(
                                ps_a[:], w1k[:, off : off + 128], xk, start=st, stop=sp
                            )
                            nc.tensor.matmul(
                                ps_b[:], w1k[:, off + 128 : off + 256], xk,
                                start=st, stop=sp,
                            )
                        tmp = tmpp.tile([128, CHUNK], f32, tag="tmp")
                        nc.scalar.activation(tmp[:], ps_a[:], SILU)
                        a = actp.tile([128, CHUNK], bf16, tag=f"act_{i}_{j}")
                        act_t[i][j] = a
                        nc.vector.tensor_mul(a[:], tmp[:], ps_b[:])
                    if S == 0 and t_s < len(scalar_batches):
                        for t, src in scalar_batches[t_s]:
                            nc.scalar.dma_start(out=t[:], in_=src)

                # mm2: out[t, h], 8 m-tiles per super-chunk. k-outer/
                # n-inner keeps 4 PSUM banks accumulating; the very last
                # m-tile flips to n-outer so each bank finishes early and
                # its copy + store overlap the remaining matmuls.
                for m in range(NT):
                    i, mc = divmod(m, NT // NCI)
                    last = (S == NSC - 1) and (m == NT - 1)
                    po = [
                        psp.tile([128, 512], f32, tag="ps", name=f"po_{S}_{m}_{n}")
                        for n in range(NHO)
                    ]
                    if last:
                        for n in range(NHO):
                            for k in range(NF):
                                nc.tensor.matmul(
                                    po[n][:],
                                    act_t[i][k][:, mc * 128 : (mc + 1) * 128],
                                    w2_t[k][:, n * 512 : (n + 1) * 512],
                                    start=(k == 0),
                                    stop=(k == NF - 1),
                                )
                    else:
                        for k in range(NF):
                            lhsT = act_t[i][k][:, mc * 128 : (mc + 1) * 128]
                            for n in range(NHO):
                                nc.tensor.matmul(
                                    po[n][:],
                                    lhsT,
                                    w2_t[k][:, n * 512 : (n + 1) * 512],
                                    start=(k == 0),
                                    stop=(k == NF - 1),
                                )
                    r0 = (S * NCI + i) * CHUNK + mc * 128
                    for n in range(NHO):
                        osb = outp.tile([128, 512], f32, tag="osb")
                        nc.scalar.copy(osb[:], po[n][:])
                        # Final m-tile's store configs ride the (idle) SP
                        # queue: fresh DMA slots + off the scalar stream,
                        # so the kernel tail isn't serialized behind them.
                        seng = nc.sync if last else nc.scalar
                        seng.dma_start(
                            out=out_d[r0 : r0 + 128, n * 512 : (n + 1) * 512],
                            in_=osb[:],
                        )
    if not nc.is_finalized():
        nc.finalize()  # Bacc.finalize runs the lowering pipeline (sem split, alloc_regs)
    return nc


def _get_nc():
    if "nc" not in _CACHE:
        _CACHE["nc"] = _build()
    return _CACHE["nc"]


def _interleave_w1(w1e: np.ndarray) -> np.ndarray:
    """[H, 2816] -> same shape with cols reordered so swiglu pair j
    (a_j = cols [128j,128j+128), b_j = cols [1408+128j, ...)) becomes the
    contiguous range [256j, 256j+256) = [a_j | b_j]."""
    a = w1e[:, :F].reshape(H, NF, 128)
    b = w1e[:, F:].reshape(H, NF, 128)
    return np.stack([a, b], axis=2).reshape(H, F2)


def kernel(permuted_hidden_states, num_tokens_per_expert, w1, w2):
    from concourse.bass_utils import run_bass_kernel_spmd

    x = np.asarray(permuted_hidden_states, dtype=np.float32)
    w1 = np.asarray(w1, dtype=np.float32)
    w2 = np.asarray(w2, dtype=np.float32)
    ntpe = np.asarray(num_tokens_per_expert)
    assert x.shape == (T_TOTAL, H) and w1.shape == (E, H, F2) and w2.shape == (E, F, H)
    # Reference semantics rely on the static equal split.
    assert np.all(ntpe == TPC), f"expected equal {TPC}-token splits, got {ntpe}"

    bf = ml_dtypes.bfloat16
    in_maps = []
    for e in range(E):
        xe = x[e * TPC : (e + 1) * TPC]
        in_maps.append(
            {
                "xT": np.ascontiguousarray(xe.T).astype(bf),
                "w1": np.ascontiguousarray(_interleave_w1(w1[e])).astype(bf),
                "w2": np.ascontiguousarray(w2[e]).astype(bf),
            }
        )

    nc = _get_nc()
    res = run_bass_kernel_spmd(nc, in_maps, list(range(E)), trace=TRACE)
    LAST["exec_time_ns"] = res.exec_time_ns
    LAST["mean_exec_time_ns"] = res.mean_exec_time_ns
    LAST["profile_json"] = res.profile_json
    out = np.concatenate([res.results[i]["out"] for i in range(E)], axis=0)
    return np.ascontiguousarray(out.astype(np.float32))


# revision 15
# speedup vs baseline: 1.0264x; 1.0264x over previous
"""MoE expert FFN (swiglu) kernel for 8 trn2 NeuronCores.

Expert parallelism: 8 experts, one per core. Each core computes, for its
expert e:
    h   = x_e @ w1_e            # [2048, 2048] @ [2048, 2816]
    act = silu(h[:, :1408]) * h[:, 1408:]
    out = act @ w2_e            # [2048, 1408] @ [1408, 2048]

Tokens arrive pre-sorted by expert with equal counts (2048/expert), so
sharding is a static slice and the gather is a concat. No collectives.

Device-side layout (all bf16 compute, fp32 PSUM accumulation, fp32 out):
  mm1: out[f, t] tiles; lhsT = w1[h,f] 128x128 tiles (stationary),
       rhs = xT[h, t] (moving, N=512) -> inter is [f, t], the layout mm2
       needs, so no on-device transpose anywhere (x is transposed on host).
  swiglu pairs: w1 columns are interleaved on HOST so pair j = cols
       [256j, 256j+256) = [a_j | b_j]; act_j = silu(a)*b via ACT(Silu)
       + DVE mul -> bf16 SBUF.
  mm2: out[t, h] tiles; lhsT = act[f, t] 128-col slices (stationary),
       rhs = w2[f, h] (moving, N=512). PSUM -> SBUF f32 -> DMA to out.

v5 scheduling. Measured DMA aggregate is only ~220-300GB/s shared by all
queues, so a 512-token first chunk consumes w1 (one 3-pair block per
20.4us) faster than it can stream -> 15-25us of PE stalls (v3/v4). Fix:
process token chunks JOINTLY in super-chunks of 1024 tokens (chunks 0+1,
then 2+3). mm1 advances one swiglu pair per FOUR psum chains (a0,b0,a1,
b1 over both 512-chunks), halving the required w1 block cadence to
40.9us, which the DMA sustains with slack:
  - sync  HWDGE: b0 k-even interleaved with x1 k-even, then b1/b2
    k-even, w2 k-even, then x2+x3 (super-chunk 1, needed ~100us later).
  - scalar HWDGE: same with k-odd, then output stores.
  - gpsimd SWDGE: x0, then w1 b3 (needed only ~150us in), then spare.
  PSUM: quad = 4 banks + 4 draining = 8. act/x SBUF slots are reused
  across super-chunks (bufs=1; Tile inserts the release waits).
  mm2 runs per super-chunk (8 m-tiles); the very last m-tile is n-outer
  so its PSUM banks drain while the PE finishes -> shorter tail.

Weights stay resident in SBUF (bf16: 88KB + 44KB per partition).
PE-bound: ~451us of matmul per core at 2.4GHz; target is wall ~= that.
"""

import os
import sys

sys.path.insert(0, "/opt/trn_rl_repo")

import numpy as np
import ml_dtypes

E = 8             # experts == cores
T_TOTAL = 16384
H = 2048
F = 1408
F2 = 2 * F        # 2816
TPC = T_TOTAL // E  # 2048 tokens per core
CHUNK = 512
NSC = 2                     # super-chunks
NCI = 2                     # chunks per super-chunk
KH = H // 128               # 16 contraction tiles for mm1
NF = F // 128               # 11 swiglu pairs
NT = (NCI * CHUNK) // 128   # 8 m-tiles per super-chunk in mm2
NHO = H // 512              # 4 output column blocks

# w1 column blocks (in interleaved-pair space). The first three pairs get
# their own 256-col tile so quad j only waits for pair j's 16 k-slices
# during the DMA-paced startup; later pairs use 768/512-col blocks (fewer
# DMAs, arrive with slack).
W1_BLOCKS = [(0, 1), (1, 1), (2, 1), (3, 3), (6, 3), (9, 2)]  # (first pair, n)
W1_PAIR_BLOCK = [0, 1, 2, 3, 3, 3, 4, 4, 4, 5, 5]  # pair j -> block index

_CACHE = {}

# Optional knobs read by test.py (not used by the grading harness).
TRACE = os.environ.get("BASS_TRACE_KERNEL", "0") == "1"
LAST = {}


def _build():
    from concourse import bacc, tile, mybir

    bf16 = mybir.dt.bfloat16
    f32 = mybir.dt.float32
    SILU = mybir.ActivationFunctionType.Silu

    # Bacc (not plain Bass): its lowering pipeline splits multi-sem waits
    # into EventSemaphore pairs — TRN2 allows at most 1 wait per instruction.
    nc = bacc.Bacc()
    xT_d = nc.declare_dram_parameter("xT", [H, TPC], bf16, isOutput=False)
    w1_d = nc.declare_dram_parameter("w1", [H, F2], bf16, isOutput=False)
    w2_d = nc.declare_dram_parameter("w2", [F, H], bf16, isOutput=False)
    out_d = nc.declare_dram_parameter("out", [TPC, H], f32, isOutput=True)

    def x_dram(c, k):
        return xT_d[k * 128 : (k + 1) * 128, c * CHUNK : (c + 1) * CHUNK]

    with tile.TileContext(nc) as tc:
        with (
            tc.tile_pool(name="w1p", bufs=1) as w1p,
            tc.tile_pool(name="w2p", bufs=1) as w2p,
            tc.tile_pool(name="xp", bufs=1) as xp,
            tc.tile_pool(name="actp", bufs=1) as actp,
            tc.tile_pool(name="tmpp", bufs=2) as tmpp,
            tc.tile_pool(name="outp", bufs=4) as outp,
            tc.tile_pool(name="psp", bufs=8, space="PSUM") as psp,
        ):
            # x chunk 0 on gpsimd (SWDGE): both HWDGE queues stay free for
            # w1/x1, which gate the first swiglu quads.
            x0_t = []
            for k in range(KH):
                t = xp.tile([128, CHUNK], bf16, tag=f"x_0_{k}", name=f"x0_{k}")
                x0_t.append(t)
                nc.gpsimd.dma_start(out=t[:], in_=x_dram(0, k))

            # w1 blocks b0-b2 + x chunk 1, streamed in PE consumption order
            # on the two HWDGE queues (k-parity split). x1 k-slices are
            # interleaved into the b0 stream since quad j consumes
            # (w1_k, x0_k, x1_k) together. w1 b3 rides gpsimd after x0.
            #
            # CRITICAL scheduling constraint (cost v5 15us of PE stall):
            # the scalar HWDGE configs and the swiglu ACTIVATEs share ONE
            # sequential instruction stream, and each config beyond the
            # 4-slot rotation WAITS for an older transfer. Any scalar DMA
            # emitted before the mm1 loop therefore delays the first silu
            # (and with it the PSUM drain the PE depends on) by tens of
            # us. So scalar gets ONLY the b0-odd/x1-odd front-load here;
            # its remaining transfers (b1/b2-odd, w2-odd) are emitted
            # inside the mm1 loop in batches of <=4 (the slot count) after
            # each quad's swiglu, where the waits are long satisfied.
            w1_t = [[None] * len(W1_BLOCKS) for _ in range(KH)]
            x1_t = []
            for k in range(KH):
                t = xp.tile([128, CHUNK], bf16, tag=f"x_1_{k}", name=f"x1_{k}")
                x1_t.append(t)
            # Issue order is load-bearing: each queue executes configs in
            # program order, so emission order IS arrival order. sync gets
            # p0e..p2e | x1e | b1e b2e | w2e | x2 x3; scalar gets
            # p0o..p2o | x1o | (batches inside mm1); gpsimd x0 | b3.
            w1_dma = {}  # (b, k) -> (tile, dram slice), deferred issue
            sync_late = []  # b1/b2 k-even, behind x1 on the sync queue
            for b, (p0, npair) in enumerate(W1_BLOCKS):
                c0, cw = p0 * 256, npair * 256
                for k in range(KH):
                    t = w1p.tile([128, cw], bf16, tag=f"w1_{k}_{b}")
                    w1_t[k][b] = t
                    src = w1_d[k * 128 : (k + 1) * 128, c0 : c0 + cw]
                    if b == len(W1_BLOCKS) - 1:
                        nc.gpsimd.dma_start(out=t[:], in_=src)
                    elif b <= 2:
                        eng = nc.sync if k % 2 == 0 else nc.scalar
                        eng.dma_start(out=t[:], in_=src)
                    elif k % 2 == 0:
                        sync_late.append((t, src))
                    else:
                        w1_dma[(b, k)] = (t, src)
            # x chunk 1 rides both HWDGE queues AFTER the first three w1
            # pairs: the stagger below delays its first use to step 2
            # (~34us), so it must not compete with p0-p2 in the front-load.
            for k in range(KH):
                eng = nc.sync if k % 2 == 0 else nc.scalar
                eng.dma_start(out=x1_t[k][:], in_=x_dram(1, k))
            for t, src in sync_late:
                nc.sync.dma_start(out=t[:], in_=src)
            # Resident w2: 11 tiles [128, 2048]; even k on sync after w1,
            # odd k deferred into the mm1 loop (scalar) — not needed until
            # mm2 of super-chunk 0 (~190us in).
            w2_t = []
            w2_dma = []
            for k in range(NF):
                t = w2p.tile([128, H], bf16, tag=f"w2_{k}")
                w2_t.append(t)
                src = w2_d[k * 128 : (k + 1) * 128, :]
                if k % 2 == 0:
                    nc.sync.dma_start(out=t[:], in_=src)
                else:
                    w2_dma.append((t, src))
            # Scalar-queue batches (<=4 configs each) emitted after step
            # t's swiglu in super-chunk 0: 768-block odds, then w2-odd.
            deferred = [w1_dma[(b, k)] for b in (3, 4) for k in range(1, KH, 2)]
            deferred += w2_dma
            scalar_batches = [deferred[i : i + 4] for i in range(0, len(deferred), 4)]

            for S in range(NSC):
                if S == 0:
                    x_t = [x0_t, x1_t]
                else:
                    # Chunks 2+3 reuse chunk 0+1's SBUF slots (released at
                    # the end of super-chunk 0's mm1); the SP queue is idle
                    # from ~90us so the waits cost nothing.
                    x_t = [[], []]
                    for i in range(NCI):
                        for k in range(KH):
                            t = xp.tile(
                                [128, CHUNK], bf16, tag=f"x_{i}_{k}",
                                name=f"x_{S}_{i}_{k}",
                            )
                            x_t[i].append(t)
                            nc.sync.dma_start(out=t[:], in_=x_dram(S * NCI + i, k))

                # mm1 + swiglu, software-staggered: step t runs pair t of
                # chunk 0 and pair t-2 of chunk 1. Chunk 1's weights are
                # always two pairs old (resident), so only (x0, p0..p2)
                # are on the DMA critical path at startup — x1 isn't
                # needed until step 2 (~34us). Weight demand per block
                # still averages half the v3 rate, which the shared
                # ~290GB/s DMA sustains without PE stalls.
                STAG = 2
                act_t = [[None] * NF, [None] * NF]  # [chunk][j]
                for t_s in range(NF + STAG):
                    chains = []
                    if t_s < NF:
                        chains.append((0, t_s))
                    if t_s >= STAG:
                        chains.append((1, t_s - STAG))
                    for i, j in chains:
                        b = W1_PAIR_BLOCK[j]
                        off = (j - W1_BLOCKS[b][0]) * 256
                        ps_a = psp.tile(
                            [128, CHUNK], f32, tag="ps", name=f"ps_{S}_{i}_{j}_a"
                        )
                        ps_b = psp.tile(
                            [128, CHUNK], f32, tag="ps", name=f"ps_{S}_{i}_{j}_b"
                        )
                        for k in range(KH):
                            w1k = w1_t[k][b]
                            st, sp = (k == 0), (k == KH - 1)
                            xk = x_t[i][k][:]
                            nc.tensor.matmul(
                                ps_a[:], w1k[:, off : off + 128], xk, start=st, stop=sp
                            )
                            nc.tensor.matmul(
                                ps_b[:], w1k[:, off + 128 : off + 256], xk,
                                start=st, stop=sp,
                            )
                        tmp = tmpp.tile([128, CHUNK], f32, tag="tmp")
                        nc.scalar.activation(tmp[:], ps_a[:], SILU)
                        a = actp.tile([128, CHUNK], bf16, tag=f"act_{i}_{j}")
                        act_t[i][j] = a
                        nc.vector.tensor_mul(a[:], tmp[:], ps_b[:])
                    if S == 0 and t_s < len(scalar_batches):
                        for t, src in scalar_batches[t_s]:
                            nc.scalar.dma_start(out=t[:], in_=src)

                # mm2: out[t, h], 8 m-tiles per super-chunk. k-outer/
                # n-inner keeps 4 PSUM banks accumulating; the very last
                # m-tile flips to n-outer so each bank finishes early and
                # its copy + store overlap the remaining matmuls.
                for m in range(NT):
                    i, mc = divmod(m, NT // NCI)
                    last = (S == NSC - 1) and (m == NT - 1)
                    po = [
                        psp.tile([128, 512], f32, tag="ps", name=f"po_{S}_{m}_{n}")
                        for n in range(NHO)
                    ]
                    if last:
                        for n in range(NHO):
                            for k in range(NF):
                                nc.tensor.matmul(
                                    po[n][:],
                                    act_t[i][k][:, mc * 128 : (mc + 1) * 128],
                                    w2_t[k][:, n * 512 : (n + 1) * 512],
                                    start=(k == 0),
                                    stop=(k == NF - 1),
                                )
                    else:
                        for k in range(NF):
                            lhsT = act_t[i][k][:, mc * 128 : (mc + 1) * 128]
                            for n in range(NHO):
                                nc.tensor.matmul(
                                    po[n][:],
                                    lhsT,
                                    w2_t[k][:, n * 512 : (n + 1) * 512],
                                    start=(k == 0),
                                    stop=(k == NF - 1),
                                )
                    r0 = (S * NCI + i) * CHUNK + mc * 128
                    for n in range(NHO):
                        osb = outp.tile([128, 512], f32, tag="osb")
                        nc.scalar.copy(osb[:], po[n][:])
                        # Final m-tile's store configs ride the (idle) SP
                        # queue: fresh DMA slots + off the scalar stream,
                        # so the kernel tail isn't serialized behind them.
                        seng = nc.sync if last else nc.scalar
                        seng.dma_start(
                            out=out_d[r0 : r0 + 128, n * 512 : (n + 1) * 512],
                            in_=osb[:],
                        )
    if not nc.is_finalized():
        nc.finalize()  # Bacc.finalize runs the lowering pipeline (sem split, alloc_regs)
    return nc


def _get_nc():
    if "nc" not in _CACHE:
        _CACHE["nc"] = _build()
    return _CACHE["nc"]


def _interleave_w1(w1e: np.ndarray) -> np.ndarray:
    """[H, 2816] -> same shape with cols reordered so swiglu pair j
    (a_j = cols [128j,128j+128), b_j = cols [1408+128j, ...)) becomes the
    contiguous range [256j, 256j+256) = [a_j | b_j]."""
    a = w1e[:, :F].reshape(H, NF, 128)
    b = w1e[:, F:].reshape(H, NF, 128)
    return np.stack([a, b], axis=2).reshape(H, F2)


def kernel(permuted_hidden_states, num_tokens_per_expert, w1, w2):
    from concourse.bass_utils import run_bass_kernel_spmd

    x = np.asarray(permuted_hidden_states, dtype=np.float32)
    w1 = np.asarray(w1, dtype=np.float32)
    w2 = np.asarray(w2, dtype=np.float32)
    ntpe = np.asarray(num_tokens_per_expert)
    assert x.shape == (T_TOTAL, H) and w1.shape == (E, H, F2) and w2.shape == (E, F, H)
    # Reference semantics rely on the static equal split.
    assert np.all(ntpe == TPC), f"expected equal {TPC}-token splits, got {ntpe}"

    bf = ml_dtypes.bfloat16
    in_maps = []
    for e in range(E):
        xe = x[e * TPC : (e + 1) * TPC]
        in_maps.append(
            {
                "xT": np.ascontiguousarray(xe.T).astype(bf),
                "w1": np.ascontiguousarray(_interleave_w1(w1[e])).astype(bf),
                "w2": np.ascontiguousarray(w2[e]).astype(bf),
            }
        )

    nc = _get_nc()
    res = run_bass_kernel_spmd(nc, in_maps, list(range(E)), trace=TRACE)
    LAST["exec_time_ns"] = res.exec_time_ns
    LAST["mean_exec_time_ns"] = res.mean_exec_time_ns
    LAST["profile_json"] = res.profile_json
    out = np.concatenate([res.results[i]["out"] for i in range(E)], axis=0)
    return np.ascontiguousarray(out.astype(np.float32))


# revision 22
# speedup vs baseline: 1.0326x; 1.0061x over previous
"""MoE expert FFN (swiglu) kernel for 8 trn2 NeuronCores.

Expert parallelism: 8 experts, one per core. Each core computes, for its
expert e:
    h   = x_e @ w1_e            # [2048, 2048] @ [2048, 2816]
    act = silu(h[:, :1408]) * h[:, 1408:]
    out = act @ w2_e            # [2048, 1408] @ [1408, 2048]

Tokens arrive pre-sorted by expert with equal counts (2048/expert), so
sharding is a static slice and the gather is a concat. No collectives.

Device-side layout (all bf16 compute, fp32 PSUM accumulation, fp32 out):
  mm1: out[f, t] tiles; lhsT = w1[h,f] 128x128 tiles (stationary),
       rhs = xT[h, t] (moving, N=512) -> inter is [f, t], the layout mm2
       needs, so no on-device transpose anywhere (x is transposed on host).
  swiglu pairs: w1 columns are interleaved on HOST so pair j = cols
       [256j, 256j+256) = [a_j | b_j]; act_j = silu(a)*b via ACT(Silu)
       + DVE mul -> bf16 SBUF.
  mm2: out[t, h] tiles; lhsT = act[f, t] 128-col slices (stationary),
       rhs = w2[f, h] (moving, N=512). PSUM -> SBUF f32 -> DMA to out.

v5 scheduling. Measured DMA aggregate is only ~220-300GB/s shared by all
queues, so a 512-token first chunk consumes w1 (one 3-pair block per
20.4us) faster than it can stream -> 15-25us of PE stalls (v3/v4). Fix:
process token chunks JOINTLY in super-chunks of 1024 tokens (chunks 0+1,
then 2+3). mm1 advances one swiglu pair per FOUR psum chains (a0,b0,a1,
b1 over both 512-chunks), halving the required w1 block cadence to
40.9us, which the DMA sustains with slack:
  - sync  HWDGE: b0 k-even interleaved with x1 k-even, then b1/b2
    k-even, w2 k-even, then x2+x3 (super-chunk 1, needed ~100us later).
  - scalar HWDGE: same with k-odd, then output stores.
  - gpsimd SWDGE: x0, then w1 b3 (needed only ~150us in), then spare.
  PSUM: quad = 4 banks + 4 draining = 8. act/x SBUF slots are reused
  across super-chunks (bufs=1; Tile inserts the release waits).
  mm2 runs per super-chunk (8 m-tiles); the very last m-tile is n-outer
  so its PSUM banks drain while the PE finishes -> shorter tail.

Weights stay resident in SBUF (bf16: 88KB + 44KB per partition).
PE-bound: ~451us of matmul per core at 2.4GHz; target is wall ~= that.
"""

import os
import sys

sys.path.insert(0, "/opt/trn_rl_repo")

import numpy as np
import ml_dtypes

E = 8             # experts == cores
T_TOTAL = 16384
H = 2048
F = 1408
F2 = 2 * F        # 2816
TPC = T_TOTAL // E  # 2048 tokens per core
CHUNK = 512
NSC = 2                     # super-chunks
NCI = 2                     # chunks per super-chunk
KH = H // 128               # 16 contraction tiles for mm1
NF = F // 128               # 11 swiglu pairs
NT = (NCI * CHUNK) // 128   # 8 m-tiles per super-chunk in mm2
NHO = H // 512              # 4 output column blocks

# w1 column blocks (in interleaved-pair space). The first three pairs get
# their own 256-col tile so quad j only waits for pair j's 16 k-slices
# during the DMA-paced startup; later pairs use 768/512-col blocks (fewer
# DMAs, arrive with slack).
W1_BLOCKS = [(0, 1), (1, 1), (2, 1), (3, 3), (6, 3), (9, 2)]  # (first pair, n)
W1_PAIR_BLOCK = [0, 1, 2, 3, 3, 3, 4, 4, 4, 5, 5]  # pair j -> block index

_CACHE = {}

# Optional knobs read by test.py (not used by the grading harness).
TRACE = os.environ.get("BASS_TRACE_KERNEL", "0") == "1"
LAST = {}


def _build():
    from concourse import bacc, tile, mybir

    bf16 = mybir.dt.bfloat16
    f32 = mybir.dt.float32
    SILU = mybir.ActivationFunctionType.Silu

    # Bacc (not plain Bass): its lowering pipeline splits multi-sem waits
    # into EventSemaphore pairs — TRN2 allows at most 1 wait per instruction.
    nc = bacc.Bacc()
    # x is host-packed as [p, chunk, k, t] -> [128, NCH*KH*CHUNK]: any k-range
    # of one chunk is a single contiguous 2D DMA slice, so x streams in
    # k-PAIR transfers (half the configs of per-k slices).
    xT_d = nc.declare_dram_parameter(
        "xT", [128, (TPC // CHUNK) * KH * CHUNK], bf16, isOutput=False
    )
    w1_d = nc.declare_dram_parameter("w1", [H, F2], bf16, isOutput=False)
    w2_d = nc.declare_dram_parameter("w2", [F, H], bf16, isOutput=False)
    out_d = nc.declare_dram_parameter("out", [TPC, H], f32, isOutput=True)

    def x_dram_pair(c, kp):
        c0 = (c * KH + 2 * kp) * CHUNK
        return xT_d[:, c0 : c0 + 2 * CHUNK]

    with tile.TileContext(nc) as tc:
        with (
            tc.tile_pool(name="w1p", bufs=1) as w1p,
            tc.tile_pool(name="w2p", bufs=1) as w2p,
            tc.tile_pool(name="xp", bufs=1) as xp,
            tc.tile_pool(name="actp", bufs=1) as actp,
            tc.tile_pool(name="tmpp", bufs=2) as tmpp,
            tc.tile_pool(name="outp", bufs=4) as outp,
            tc.tile_pool(name="psp", bufs=8, space="PSUM") as psp,
        ):
            # x chunk 0 on gpsimd (SWDGE) in k-pair tiles: both HWDGE
            # queues stay free for w1, which gates the first swiglu steps.
            x0_t = []
            for kp in range(KH // 2):
                t = xp.tile([128, 2 * CHUNK], bf16, tag=f"x_0_{kp}", name=f"x0_{kp}")
                x0_t.append(t)
                nc.gpsimd.dma_start(out=t[:], in_=x_dram_pair(0, kp))

            # w1 blocks b0-b2 + x chunk 1, streamed in PE consumption order
            # on the two HWDGE queues (k-parity split). x1 k-slices are
            # interleaved into the b0 stream since quad j consumes
            # (w1_k, x0_k, x1_k) together. w1 b3 rides gpsimd after x0.
            #
            # CRITICAL scheduling constraint (cost v5 15us of PE stall):
            # the scalar HWDGE configs and the swiglu ACTIVATEs share ONE
            # sequential instruction stream, and each config beyond the
            # 4-slot rotation WAITS for an older transfer. Any scalar DMA
            # emitted before the mm1 loop therefore delays the first silu
            # (and with it the PSUM drain the PE depends on) by tens of
            # us. So scalar gets ONLY the b0-odd/x1-odd front-load here;
            # its remaining transfers (b1/b2-odd, w2-odd) are emitted
            # inside the mm1 loop in batches of <=4 (the slot count) after
            # each quad's swiglu, where the waits are long satisfied.
            w1_t = [[None] * len(W1_BLOCKS) for _ in range(KH)]
            x1_t = []
            for kp in range(KH // 2):
                t = xp.tile([128, 2 * CHUNK], bf16, tag=f"x_1_{kp}", name=f"x1_{kp}")
                x1_t.append(t)
            # Issue order is load-bearing: each queue executes configs in
            # program order, so emission order IS arrival order. sync gets
            # p0e..p2e | x1e | b1e b2e | w2e | x2 x3; scalar gets
            # p0o..p2o | x1o | (batches inside mm1); gpsimd x0 | b3.
            w1_dma = {}  # (b, k) -> (tile, dram slice), deferred issue
            sync_late = []  # b1/b2 k-even, behind x1 on the sync queue
            for b, (p0, npair) in enumerate(W1_BLOCKS):
                c0, cw = p0 * 256, npair * 256
                for k in range(KH):
                    t = w1p.tile([128, cw], bf16, tag=f"w1_{k}_{b}")
                    w1_t[k][b] = t
                    src = w1_d[k * 128 : (k + 1) * 128, c0 : c0 + cw]
                    if b == len(W1_BLOCKS) - 1:
                        nc.gpsimd.dma_start(out=t[:], in_=src)
                    elif b <= 2:
                        eng = nc.sync if k % 2 == 0 else nc.scalar
                        eng.dma_start(out=t[:], in_=src)
                    elif k % 2 == 0:
                        sync_late.append((t, src))
                    else:
                        w1_dma[(b, k)] = (t, src)
            # x chunk 1 rides both HWDGE queues AFTER the first three w1
            # pairs: the stagger below delays its first use to step 2
            # (~34us), so it must not compete with p0-p2 in the front-load.
            for kp in range(KH // 2):
                eng = nc.sync if kp % 2 == 0 else nc.scalar
                eng.dma_start(out=x1_t[kp][:], in_=x_dram_pair(1, kp))
            for t, src in sync_late:
                nc.sync.dma_start(out=t[:], in_=src)
            # Resident w2: 11 tiles [128, 2048]; even k on sync after w1,
            # odd k deferred into the mm1 loop (scalar) — not needed until
            # mm2 of super-chunk 0 (~190us in).
            w2_t = []
            w2_dma = []
            for k in range(NF):
                t = w2p.tile([128, H], bf16, tag=f"w2_{k}")
                w2_t.append(t)
                src = w2_d[k * 128 : (k + 1) * 128, :]
                if k % 2 == 0:
                    nc.sync.dma_start(out=t[:], in_=src)
                else:
                    w2_dma.append((t, src))
            # Scalar-queue batches (<=4 configs each) emitted after step
            # t's swiglu in super-chunk 0: 768-block odds, then w2-odd.
            deferred = [w1_dma[(b, k)] for b in (3, 4) for k in range(1, KH, 2)]
            deferred += w2_dma
            scalar_batches = [deferred[i : i + 4] for i in range(0, len(deferred), 4)]

            for S in range(NSC):
                if S == 0:
                    x_t = [x0_t, x1_t]
                else:
                    # Chunks 2+3 reuse chunk 0+1's SBUF slots (released at
                    # the end of super-chunk 0's mm1); the SP queue is idle
                    # from ~90us so the waits cost nothing.
                    x_t = [[], []]
                    for i in range(NCI):
                        for kp in range(KH // 2):
                            t = xp.tile(
                                [128, 2 * CHUNK], bf16, tag=f"x_{i}_{kp}",
                                name=f"x_{S}_{i}_{kp}",
                            )
                            x_t[i].append(t)
                            nc.sync.dma_start(
                                out=t[:], in_=x_dram_pair(S * NCI + i, kp)
                            )

                # mm1 + swiglu, software-staggered: step t runs pair t of
                # chunk 0 and pair t-2 of chunk 1. Chunk 1's weights are
                # always two pairs old (resident), so only (x0, p0..p2)
                # are on the DMA critical path at startup — x1 isn't
                # needed until step 2 (~34us). Weight demand per block
                # still averages half the v3 rate, which the shared
                # ~290GB/s DMA sustains without PE stalls.
                STAG = 2
                act_t = [[None] * NF, [None] * NF]  # [chunk][j]
                for t_s in range(NF + STAG):
                    chains = []
                    if t_s < NF:
                        chains.append((0, t_s))
                    if t_s >= STAG:
                        chains.append((1, t_s - STAG))
                    for i, j in chains:
                        b = W1_PAIR_BLOCK[j]
                        off = (j - W1_BLOCKS[b][0]) * 256
                        ps_a = psp.tile(
                            [128, CHUNK], f32, tag="ps", name=f"ps_{S}_{i}_{j}_a"
                        )
                        ps_b = psp.tile(
                            [128, CHUNK], f32, tag="ps", name=f"ps_{S}_{i}_{j}_b"
                        )
                        for k in range(KH):
                            w1k = w1_t[k][b]
                            st, sp = (k == 0), (k == KH - 1)
                            xk = x_t[i][k // 2][:, (k % 2) * CHUNK : (k % 2 + 1) * CHUNK]
                            nc.tensor.matmul(
                                ps_a[:], w1k[:, off : off + 128], xk, start=st, stop=sp
                            )
                            nc.tensor.matmul(
                                ps_b[:], w1k[:, off + 128 : off + 256], xk,
                                start=st, stop=sp,
                            )
                        tmp = tmpp.tile([128, CHUNK], f32, tag="tmp")
                        nc.scalar.activation(tmp[:], ps_a[:], SILU)
                        a = actp.tile([128, CHUNK], bf16, tag=f"act_{i}_{j}")
                        act_t[i][j] = a
                        nc.vector.tensor_mul(a[:], tmp[:], ps_b[:])
                    if S == 0 and t_s < len(scalar_batches):
                        for t, src in scalar_batches[t_s]:
                            nc.scalar.dma_start(out=t[:], in_=src)

                # mm2: out[t, h], 8 m-tiles per super-chunk. k-outer/
                # n-inner keeps 4 PSUM banks accumulating; the very last
                # m-tile flips to n-outer so each bank finishes early and
                # its copy + store overlap the remaining matmuls.
                for m in range(NT):
                    i, mc = divmod(m, NT // NCI)
                    last = (S == NSC - 1) and (m == NT - 1)
                    po = [
                        psp.tile([128, 512], f32, tag="ps", name=f"po_{S}_{m}_{n}")
                        for n in range(NHO)
                    ]
                    if last:
                        for n in range(NHO):
                            for k in range(NF):
                                nc.tensor.matmul(
                                    po[n][:],
                                    act_t[i][k][:, mc * 128 : (mc + 1) * 128],
                                    w2_t[k][:, n * 512 : (n + 1) * 512],
                                    start=(k == 0),
                                    stop=(k == NF - 1),
                                )
                    else:
                        for k in range(NF):
                            lhsT = act_t[i][k][:, mc * 128 : (mc + 1) * 128]
                            for n in range(NHO):
                                nc.tensor.matmul(
                                    po[n][:],
                                    lhsT,
                                    w2_t[k][:, n * 512 : (n + 1) * 512],
                                    start=(k == 0),
                                    stop=(k == NF - 1),
                                )
                    r0 = (S * NCI + i) * CHUNK + mc * 128
                    for n in range(NHO):
                        osb = outp.tile([128, 512], f32, tag="osb")
                        nc.scalar.copy(osb[:], po[n][:])
                        # Final m-tile's store configs ride the (idle) SP
                        # queue: fresh DMA slots + off the scalar stream,
                        # so the kernel tail isn't serialized behind them.
                        seng = nc.sync if last else nc.scalar
                        seng.dma_start(
                            out=out_d[r0 : r0 + 128, n * 512 : (n + 1) * 512],
                            in_=osb[:],
                        )
    if not nc.is_finalized():
        nc.finalize()  # Bacc.finalize runs the lowering pipeline (sem split, alloc_regs)
    return nc


def _get_nc():
    if "nc" not in _CACHE:
        _CACHE["nc"] = _build()
    return _CACHE["nc"]


def _interleave_w1(w1e: np.ndarray) -> np.ndarray:
    """[H, 2816] -> same shape with cols reordered so swiglu pair j
    (a_j = cols [128j,128j+128), b_j = cols [1408+128j, ...)) becomes the
    contiguous range [256j, 256j+256) = [a_j | b_j]."""
    a = w1e[:, :F].reshape(H, NF, 128)
    b = w1e[:, F:].reshape(H, NF, 128)
    return np.stack([a, b], axis=2).reshape(H, F2)


def kernel(permuted_hidden_states, num_tokens_per_expert, w1, w2):
    from concourse.bass_utils import run_bass_kernel_spmd

    x = np.asarray(permuted_hidden_states, dtype=np.float32)
    w1 = np.asarray(w1, dtype=np.float32)
    w2 = np.asarray(w2, dtype=np.float32)
    ntpe = np.asarray(num_tokens_per_expert)
    assert x.shape == (T_TOTAL, H) and w1.shape == (E, H, F2) and w2.shape == (E, F, H)
    # Reference semantics rely on the static equal split.
    assert np.all(ntpe == TPC), f"expected equal {TPC}-token splits, got {ntpe}"

    bf = ml_dtypes.bfloat16
    in_maps = []
    NCH = TPC // CHUNK
    for e in range(E):
        xe = x[e * TPC : (e + 1) * TPC]
        # pack as [p, chunk, k, t] so any k-range of a chunk is one
        # contiguous 2D DMA slice (see xT_d comment in _build).
        xg = (
            xe.reshape(NCH, CHUNK, KH, 128)
            .transpose(3, 0, 2, 1)
            .reshape(128, NCH * KH * CHUNK)
        )
        in_maps.append(
            {
                "xT": np.ascontiguousarray(xg).astype(bf),
                "w1": np.ascontiguousarray(_interleave_w1(w1[e])).astype(bf),
                "w2": np.ascontiguousarray(w2[e]).astype(bf),
            }
        )

    nc = _get_nc()
    res = run_bass_kernel_spmd(nc, in_maps, list(range(E)), trace=TRACE)
    LAST["exec_time_ns"] = res.exec_time_ns
    LAST["mean_exec_time_ns"] = res.mean_exec_time_ns
    LAST["profile_json"] = res.profile_json
    out = np.concatenate([res.results[i]["out"] for i in range(E)], axis=0)
    return np.ascontiguousarray(out.astype(np.float32))


# revision 24
# speedup vs baseline: 1.0333x; 1.0007x over previous
"""MoE expert FFN (swiglu) kernel for 8 trn2 NeuronCores.

Expert parallelism: 8 experts, one per core. Each core computes, for its
expert e:
    h   = x_e @ w1_e            # [2048, 2048] @ [2048, 2816]
    act = silu(h[:, :1408]) * h[:, 1408:]
    out = act @ w2_e            # [2048, 1408] @ [1408, 2048]

Tokens arrive pre-sorted by expert with equal counts (2048/expert), so
sharding is a static slice and the gather is a concat. No collectives.

Device-side layout (all bf16 compute, fp32 PSUM accumulation, fp32 out):
  mm1: out[f, t] tiles; lhsT = w1[h,f] 128x128 tiles (stationary),
       rhs = xT[h, t] (moving, N=512) -> inter is [f, t], the layout mm2
       needs, so no on-device transpose anywhere (x is transposed on host).
  swiglu pairs: w1 columns are interleaved on HOST so pair j = cols
       [256j, 256j+256) = [a_j | b_j]; act_j = silu(a)*b via ACT(Silu)
       + DVE mul -> bf16 SBUF.
  mm2: out[t, h] tiles; lhsT = act[f, t] 128-col slices (stationary),
       rhs = w2[f, h] (moving, N=512). PSUM -> SBUF f32 -> DMA to out.

v5 scheduling. Measured DMA aggregate is only ~220-300GB/s shared by all
queues, so a 512-token first chunk consumes w1 (one 3-pair block per
20.4us) faster than it can stream -> 15-25us of PE stalls (v3/v4). Fix:
process token chunks JOINTLY in super-chunks of 1024 tokens (chunks 0+1,
then 2+3). mm1 advances one swiglu pair per FOUR psum chains (a0,b0,a1,
b1 over both 512-chunks), halving the required w1 block cadence to
40.9us, which the DMA sustains with slack:
  - sync  HWDGE: b0 k-even interleaved with x1 k-even, then b1/b2
    k-even, w2 k-even, then x2+x3 (super-chunk 1, needed ~100us later).
  - scalar HWDGE: same with k-odd, then output stores.
  - gpsimd SWDGE: x0, then w1 b3 (needed only ~150us in), then spare.
  PSUM: quad = 4 banks + 4 draining = 8. act/x SBUF slots are reused
  across super-chunks (bufs=1; Tile inserts the release waits).
  mm2 runs per super-chunk (8 m-tiles); the very last m-tile is n-outer
  so its PSUM banks drain while the PE finishes -> shorter tail.

Weights stay resident in SBUF (bf16: 88KB + 44KB per partition).
PE-bound: ~451us of matmul per core at 2.4GHz; target is wall ~= that.
"""

import os
import sys

sys.path.insert(0, "/opt/trn_rl_repo")

import numpy as np
import ml_dtypes

E = 8             # experts == cores
T_TOTAL = 16384
H = 2048
F = 1408
F2 = 2 * F        # 2816
TPC = T_TOTAL // E  # 2048 tokens per core
CHUNK = 512
NSC = 2                     # super-chunks
NCI = 2                     # chunks per super-chunk
KH = H // 128               # 16 contraction tiles for mm1
NF = F // 128               # 11 swiglu pairs
NT = (NCI * CHUNK) // 128   # 8 m-tiles per super-chunk in mm2
NHO = H // 512              # 4 output column blocks

# w1 column blocks (in interleaved-pair space). The first three pairs get
# their own 256-col tile so quad j only waits for pair j's 16 k-slices
# during the DMA-paced startup; later pairs use 768/512-col blocks (fewer
# DMAs, arrive with slack).
W1_BLOCKS = [(0, 1), (1, 1), (2, 1), (3, 3), (6, 3), (9, 2)]  # (first pair, n)
W1_PAIR_BLOCK = [0, 1, 2, 3, 3, 3, 4, 4, 4, 5, 5]  # pair j -> block index

_CACHE = {}

# Optional knobs read by test.py (not used by the grading harness).
TRACE = os.environ.get("BASS_TRACE_KERNEL", "0") == "1"
LAST = {}


def _build():
    from concourse import bacc, tile, mybir

    bf16 = mybir.dt.bfloat16
    f32 = mybir.dt.float32
    SILU = mybir.ActivationFunctionType.Silu

    # Bacc (not plain Bass): its lowering pipeline splits multi-sem waits
    # into EventSemaphore pairs — TRN2 allows at most 1 wait per instruction.
    nc = bacc.Bacc()
    # x is host-packed as [p, chunk, k, t] -> [128, NCH*KH*CHUNK]: any k-range
    # of one chunk is a single contiguous 2D DMA slice, so x streams in
    # k-PAIR transfers (half the configs of per-k slices).
    xT_d = nc.declare_dram_parameter(
        "xT", [128, (TPC // CHUNK) * KH * CHUNK], bf16, isOutput=False
    )
    w1_d = nc.declare_dram_parameter("w1", [H, F2], bf16, isOutput=False)
    w2_d = nc.declare_dram_parameter("w2", [F, H], bf16, isOutput=False)
    # bf16 output (host upcasts): halves store bytes + the kernel-tail
    # drain of the final stores. Adds ~0.3% rounding noise on top of the
    # 0.41% bf16-matmul noise — far inside the 2e-2 gate.
    out_d = nc.declare_dram_parameter("out", [TPC, H], bf16, isOutput=True)

    def x_dram_pair(c, kp):
        c0 = (c * KH + 2 * kp) * CHUNK
        return xT_d[:, c0 : c0 + 2 * CHUNK]

    with tile.TileContext(nc) as tc:
        with (
            tc.tile_pool(name="w1p", bufs=1) as w1p,
            tc.tile_pool(name="w2p", bufs=1) as w2p,
            tc.tile_pool(name="xp", bufs=1) as xp,
            tc.tile_pool(name="actp", bufs=1) as actp,
            tc.tile_pool(name="tmpp", bufs=2) as tmpp,
            tc.tile_pool(name="outp", bufs=4) as outp,
            tc.tile_pool(name="psp", bufs=8, space="PSUM") as psp,
        ):
            # x chunk 0 on gpsimd (SWDGE) in k-pair tiles: both HWDGE
            # queues stay free for w1, which gates the first swiglu steps.
            x0_t = []
            for kp in range(KH // 2):
                t = xp.tile([128, 2 * CHUNK], bf16, tag=f"x_0_{kp}", name=f"x0_{kp}")
                x0_t.append(t)
                nc.gpsimd.dma_start(out=t[:], in_=x_dram_pair(0, kp))

            # w1 blocks b0-b2 + x chunk 1, streamed in PE consumption order
            # on the two HWDGE queues (k-parity split). x1 k-slices are
            # interleaved into the b0 stream since quad j consumes
            # (w1_k, x0_k, x1_k) together. w1 b3 rides gpsimd after x0.
            #
            # CRITICAL scheduling constraint (cost v5 15us of PE stall):
            # the scalar HWDGE configs and the swiglu ACTIVATEs share ONE
            # sequential instruction stream, and each config beyond the
            # 4-slot rotation WAITS for an older transfer. Any scalar DMA
            # emitted before the mm1 loop therefore delays the first silu
            # (and with it the PSUM drain the PE depends on) by tens of
            # us. So scalar gets ONLY the b0-odd/x1-odd front-load here;
            # its remaining transfers (b1/b2-odd, w2-odd) are emitted
            # inside the mm1 loop in batches of <=4 (the slot count) after
            # each quad's swiglu, where the waits are long satisfied.
            w1_t = [[None] * len(W1_BLOCKS) for _ in range(KH)]
            x1_t = []
            for kp in range(KH // 2):
                t = xp.tile([128, 2 * CHUNK], bf16, tag=f"x_1_{kp}", name=f"x1_{kp}")
                x1_t.append(t)
            # Issue order is load-bearing: each queue executes configs in
            # program order, so emission order IS arrival order. sync gets
            # p0e..p2e | x1e | b1e b2e | w2e | x2 x3; scalar gets
            # p0o..p2o | x1o | (batches inside mm1); gpsimd x0 | b3.
            w1_dma = {}  # (b, k) -> (tile, dram slice), deferred issue
            sync_late = []  # b1/b2 k-even, behind x1 on the sync queue
            for b, (p0, npair) in enumerate(W1_BLOCKS):
                c0, cw = p0 * 256, npair * 256
                for k in range(KH):
                    t = w1p.tile([128, cw], bf16, tag=f"w1_{k}_{b}")
                    w1_t[k][b] = t
                    src = w1_d[k * 128 : (k + 1) * 128, c0 : c0 + cw]
                    if b == len(W1_BLOCKS) - 1:
                        nc.gpsimd.dma_start(out=t[:], in_=src)
                    elif b <= 2:
                        eng = nc.sync if k % 2 == 0 else nc.scalar
                        eng.dma_start(out=t[:], in_=src)
                    elif k % 2 == 0:
                        sync_late.append((t, src))
                    else:
                        w1_dma[(b, k)] = (t, src)
            # x chunk 1 rides both HWDGE queues AFTER the first three w1
            # pairs: the stagger below delays its first use to step 2
            # (~34us), so it must not compete with p0-p2 in the front-load.
            for kp in range(KH // 2):
                eng = nc.sync if kp % 2 == 0 else nc.scalar
                eng.dma_start(out=x1_t[kp][:], in_=x_dram_pair(1, kp))
            for t, src in sync_late:
                nc.sync.dma_start(out=t[:], in_=src)
            # Resident w2: 11 tiles [128, 2048]; even k on sync after w1,
            # odd k deferred into the mm1 loop (scalar) — not needed until
            # mm2 of super-chunk 0 (~190us in).
            w2_t = []
            w2_dma = []
            for k in range(NF):
                t = w2p.tile([128, H], bf16, tag=f"w2_{k}")
                w2_t.append(t)
                src = w2_d[k * 128 : (k + 1) * 128, :]
                if k % 2 == 0:
                    nc.sync.dma_start(out=t[:], in_=src)
                else:
                    w2_dma.append((t, src))
            # Scalar-queue batches (<=4 configs each) emitted after step
            # t's swiglu in super-chunk 0: 768-block odds, then w2-odd.
            deferred = [w1_dma[(b, k)] for b in (3, 4) for k in range(1, KH, 2)]
            deferred += w2_dma
            scalar_batches = [deferred[i : i + 4] for i in range(0, len(deferred), 4)]

            for S in range(NSC):
                if S == 0:
                    x_t = [x0_t, x1_t]
                else:
                    # Chunks 2+3 reuse chunk 0+1's SBUF slots (released at
                    # the end of super-chunk 0's mm1); the SP queue is idle
                    # from ~90us so the waits cost nothing.
                    x_t = [[], []]
                    for i in range(NCI):
                        for kp in range(KH // 2):
                            t = xp.tile(
                                [128, 2 * CHUNK], bf16, tag=f"x_{i}_{kp}",
                                name=f"x_{S}_{i}_{kp}",
                            )
                            x_t[i].append(t)
                            nc.sync.dma_start(
                                out=t[:], in_=x_dram_pair(S * NCI + i, kp)
                            )

                # mm1 + swiglu, software-staggered: step t runs pair t of
                # chunk 0 and pair t-2 of chunk 1. Chunk 1's weights are
                # always two pairs old (resident), so only (x0, p0..p2)
                # are on the DMA critical path at startup — x1 isn't
                # needed until step 2 (~34us). Weight demand per block
                # still averages half the v3 rate, which the shared
                # ~290GB/s DMA sustains without PE stalls.
                STAG = 2
                act_t = [[None] * NF, [None] * NF]  # [chunk][j]
                for t_s in range(NF + STAG):
                    chains = []
                    if t_s < NF:
                        chains.append((0, t_s))
                    if t_s >= STAG:
                        chains.append((1, t_s - STAG))
                    for i, j in chains:
                        b = W1_PAIR_BLOCK[j]
                        off = (j - W1_BLOCKS[b][0]) * 256
                        ps_a = psp.tile(
                            [128, CHUNK], f32, tag="ps", name=f"ps_{S}_{i}_{j}_a"
                        )
                        ps_b = psp.tile(
                            [128, CHUNK], f32, tag="ps", name=f"ps_{S}_{i}_{j}_b"
                        )
                        for k in range(KH):
                            w1k = w1_t[k][b]
                            st, sp = (k == 0), (k == KH - 1)
                            xk = x_t[i][k // 2][:, (k % 2) * CHUNK : (k % 2 + 1) * CHUNK]
                            nc.tensor.matmul(
                                ps_a[:], w1k[:, off : off + 128], xk, start=st, stop=sp
                            )
                            nc.tensor.matmul(
                                ps_b[:], w1k[:, off + 128 : off + 256], xk,
                                start=st, stop=sp,
                            )
                        tmp = tmpp.tile([128, CHUNK], f32, tag="tmp")
                        nc.scalar.activation(tmp[:], ps_a[:], SILU)
                        a = actp.tile([128, CHUNK], bf16, tag=f"act_{i}_{j}")
                        act_t[i][j] = a
                        nc.vector.tensor_mul(a[:], tmp[:], ps_b[:])
                    if S == 0 and t_s < len(scalar_batches):
                        for t, src in scalar_batches[t_s]:
                            nc.scalar.dma_start(out=t[:], in_=src)

                # mm2: out[t, h], 8 m-tiles per super-chunk. k-outer/
                # n-inner keeps 4 PSUM banks accumulating; the very last
                # m-tile flips to n-outer so each bank finishes early and
                # its copy + store overlap the remaining matmuls.
                for m in range(NT):
                    i, mc = divmod(m, NT // NCI)
                    last = (S == NSC - 1) and (m == NT - 1)
                    po = [
                        psp.tile([128, 512], f32, tag="ps", name=f"po_{S}_{m}_{n}")
                        for n in range(NHO)
                    ]
                    if last:
                        for n in range(NHO):
                            for k in range(NF):
                                nc.tensor.matmul(
                                    po[n][:],
                                    act_t[i][k][:, mc * 128 : (mc + 1) * 128],
                                    w2_t[k][:, n * 512 : (n + 1) * 512],
                                    start=(k == 0),
                                    stop=(k == NF - 1),
                                )
                    else:
                        for k in range(NF):
                            lhsT = act_t[i][k][:, mc * 128 : (mc + 1) * 128]
                            for n in range(NHO):
                                nc.tensor.matmul(
                                    po[n][:],
                                    lhsT,
                                    w2_t[k][:, n * 512 : (n + 1) * 512],
                                    start=(k == 0),
                                    stop=(k == NF - 1),
                                )
                    r0 = (S * NCI + i) * CHUNK + mc * 128
                    for n in range(NHO):
                        osb = outp.tile([128, 512], bf16, tag="osb")
                        nc.scalar.copy(osb[:], po[n][:])
                        # Final m-tile's store configs ride the (idle) SP
                        # queue: fresh DMA slots + off the scalar stream,
                        # so the kernel tail isn't serialized behind them.
                        seng = nc.sync if last else nc.scalar
                        seng.dma_start(
                            out=out_d[r0 : r0 + 128, n * 512 : (n + 1) * 512],
                            in_=osb[:],
                        )
    if not nc.is_finalized():
        nc.finalize()  # Bacc.finalize runs the lowering pipeline (sem split, alloc_regs)
    return nc


def _get_nc():
    if "nc" not in _CACHE:
        _CACHE["nc"] = _build()
    return _CACHE["nc"]


def _interleave_w1(w1e: np.ndarray) -> np.ndarray:
    """[H, 2816] -> same shape with cols reordered so swiglu pair j
    (a_j = cols [128j,128j+128), b_j = cols [1408+128j, ...)) becomes the
    contiguous range [256j, 256j+256) = [a_j | b_j]."""
    a = w1e[:, :F].reshape(H, NF, 128)
    b = w1e[:, F:].reshape(H, NF, 128)
    return np.stack([a, b], axis=2).reshape(H, F2)


def kernel(permuted_hidden_states, num_tokens_per_expert, w1, w2):
    from concourse.bass_utils import run_bass_kernel_spmd

    x = np.asarray(permuted_hidden_states, dtype=np.float32)
    w1 = np.asarray(w1, dtype=np.float32)
    w2 = np.asarray(w2, dtype=np.float32)
    ntpe = np.asarray(num_tokens_per_expert)
    assert x.shape == (T_TOTAL, H) and w1.shape == (E, H, F2) and w2.shape == (E, F, H)
    # Reference semantics rely on the static equal split.
    assert np.all(ntpe == TPC), f"expected equal {TPC}-token splits, got {ntpe}"

    bf = ml_dtypes.bfloat16
    in_maps = []
    NCH = TPC // CHUNK
    for e in range(E):
        xe = x[e * TPC : (e + 1) * TPC]
        # pack as [p, chunk, k, t] so any k-range of a chunk is one
        # contiguous 2D DMA slice (see xT_d comment in _build).
        xg = (
            xe.reshape(NCH, CHUNK, KH, 128)
            .transpose(3, 0, 2, 1)
            .reshape(128, NCH * KH * CHUNK)
        )
        in_maps.append(
            {
                "xT": np.ascontiguousarray(xg).astype(bf),
                "w1": np.ascontiguousarray(_interleave_w1(w1[e])).astype(bf),
                "w2": np.ascontiguousarray(w2[e]).astype(bf),
            }
        )

    nc = _get_nc()
    res = run_bass_kernel_spmd(nc, in_maps, list(range(E)), trace=TRACE)
    LAST["exec_time_ns"] = res.exec_time_ns
    LAST["mean_exec_time_ns"] = res.mean_exec_time_ns
    LAST["profile_json"] = res.profile_json
    out = np.concatenate([res.results[i]["out"] for i in range(E)], axis=0)
    return np.ascontiguousarray(out.astype(np.float32))


# revision 25
# speedup vs baseline: 1.0348x; 1.0015x over previous
"""MoE expert FFN (swiglu) kernel for 8 trn2 NeuronCores.

Expert parallelism: 8 experts, one per core. Each core computes, for its
expert e:
    h   = x_e @ w1_e            # [2048, 2048] @ [2048, 2816]
    act = silu(h[:, :1408]) * h[:, 1408:]
    out = act @ w2_e            # [2048, 1408] @ [1408, 2048]

Tokens arrive pre-sorted by expert with equal counts (2048/expert), so
sharding is a static slice and the gather is a concat. No collectives.

Device-side layout (all bf16 compute, fp32 PSUM accumulation, fp32 out):
  mm1: out[f, t] tiles; lhsT = w1[h,f] 128x128 tiles (stationary),
       rhs = xT[h, t] (moving, N=512) -> inter is [f, t], the layout mm2
       needs, so no on-device transpose anywhere (x is transposed on host).
  swiglu pairs: w1 columns are interleaved on HOST so pair j = cols
       [256j, 256j+256) = [a_j | b_j]; act_j = silu(a)*b via ACT(Silu)
       + DVE mul -> bf16 SBUF.
  mm2: out[t, h] tiles; lhsT = act[f, t] 128-col slices (stationary),
       rhs = w2[f, h] (moving, N=512). PSUM -> SBUF f32 -> DMA to out.

v5 scheduling. Measured DMA aggregate is only ~220-300GB/s shared by all
queues, so a 512-token first chunk consumes w1 (one 3-pair block per
20.4us) faster than it can stream -> 15-25us of PE stalls (v3/v4). Fix:
process token chunks JOINTLY in super-chunks of 1024 tokens (chunks 0+1,
then 2+3). mm1 advances one swiglu pair per FOUR psum chains (a0,b0,a1,
b1 over both 512-chunks), halving the required w1 block cadence to
40.9us, which the DMA sustains with slack:
  - sync  HWDGE: b0 k-even interleaved with x1 k-even, then b1/b2
    k-even, w2 k-even, then x2+x3 (super-chunk 1, needed ~100us later).
  - scalar HWDGE: same with k-odd, then output stores.
  - gpsimd SWDGE: x0, then w1 b3 (needed only ~150us in), then spare.
  PSUM: quad = 4 banks + 4 draining = 8. act/x SBUF slots are reused
  across super-chunks (bufs=1; Tile inserts the release waits).
  mm2 runs per super-chunk (8 m-tiles); the very last m-tile is n-outer
  so its PSUM banks drain while the PE finishes -> shorter tail.

Weights stay resident in SBUF (bf16: 88KB + 44KB per partition).
PE-bound: ~451us of matmul per core at 2.4GHz; target is wall ~= that.
"""

import os
import sys

sys.path.insert(0, "/opt/trn_rl_repo")

import numpy as np
import ml_dtypes

E = 8             # experts == cores
T_TOTAL = 16384
H = 2048
F = 1408
F2 = 2 * F        # 2816
TPC = T_TOTAL // E  # 2048 tokens per core
CHUNK = 512
NSC = 2                     # super-chunks
NCI = 2                     # chunks per super-chunk
KH = H // 128               # 16 contraction tiles for mm1
NF = F // 128               # 11 swiglu pairs
NT = (NCI * CHUNK) // 128   # 8 m-tiles per super-chunk in mm2
NHO = H // 512              # 4 output column blocks

# w1 column blocks (in interleaved-pair space). The first three pairs get
# their own 256-col tile so quad j only waits for pair j's 16 k-slices
# during the DMA-paced startup; later pairs use 768/512-col blocks (fewer
# DMAs, arrive with slack).
W1_BLOCKS = [(0, 1), (1, 1), (2, 1), (3, 3), (6, 3), (9, 2)]  # (first pair, n)
W1_PAIR_BLOCK = [0, 1, 2, 3, 3, 3, 4, 4, 4, 5, 5]  # pair j -> block index

_CACHE = {}

# Optional knobs read by test.py (not used by the grading harness).
TRACE = os.environ.get("BASS_TRACE_KERNEL", "0") == "1"
LAST = {}


def _build():
    from concourse import bacc, tile, mybir

    bf16 = mybir.dt.bfloat16
    f32 = mybir.dt.float32
    SILU = mybir.ActivationFunctionType.Silu

    # Bacc (not plain Bass): its lowering pipeline splits multi-sem waits
    # into EventSemaphore pairs — TRN2 allows at most 1 wait per instruction.
    nc = bacc.Bacc()
    # x is host-packed as [p, chunk, k, t] -> [128, NCH*KH*CHUNK]: any k-range
    # of one chunk is a single contiguous 2D DMA slice, so x streams in
    # k-PAIR transfers (half the configs of per-k slices).
    xT_d = nc.declare_dram_parameter(
        "xT", [128, (TPC // CHUNK) * KH * CHUNK], bf16, isOutput=False
    )
    w1_d = nc.declare_dram_parameter("w1", [H, F2], bf16, isOutput=False)
    w2_d = nc.declare_dram_parameter("w2", [F, H], bf16, isOutput=False)
    # bf16 output (host upcasts): halves store bytes + the kernel-tail
    # drain of the final stores. Adds ~0.3% rounding noise on top of the
    # 0.41% bf16-matmul noise — far inside the 2e-2 gate.
    out_d = nc.declare_dram_parameter("out", [TPC, H], bf16, isOutput=True)

    def x_dram_pair(c, kp):
        c0 = (c * KH + 2 * kp) * CHUNK
        return xT_d[:, c0 : c0 + 2 * CHUNK]

    with tile.TileContext(nc) as tc:
        with (
            tc.tile_pool(name="w1p", bufs=1) as w1p,
            tc.tile_pool(name="w2p", bufs=1) as w2p,
            tc.tile_pool(name="xp", bufs=1) as xp,
            tc.tile_pool(name="actp", bufs=1) as actp,
            tc.tile_pool(name="tmpp", bufs=2) as tmpp,
            tc.tile_pool(name="outp", bufs=4) as outp,
            tc.tile_pool(name="psp", bufs=8, space="PSUM") as psp,
        ):
            # x chunk 0 on gpsimd (SWDGE) in k-pair tiles: both HWDGE
            # queues stay free for w1, which gates the first swiglu steps.
            x0_t = []
            for kp in range(KH // 2):
                t = xp.tile([128, 2 * CHUNK], bf16, tag=f"x_0_{kp}", name=f"x0_{kp}")
                x0_t.append(t)
                nc.gpsimd.dma_start(out=t[:], in_=x_dram_pair(0, kp))

            # w1 blocks b0-b2 + x chunk 1, streamed in PE consumption order
            # on the two HWDGE queues (k-parity split). x1 k-slices are
            # interleaved into the b0 stream since quad j consumes
            # (w1_k, x0_k, x1_k) together. w1 b3 rides gpsimd after x0.
            #
            # CRITICAL scheduling constraint (cost v5 15us of PE stall):
            # the scalar HWDGE configs and the swiglu ACTIVATEs share ONE
            # sequential instruction stream, and each config beyond the
            # 4-slot rotation WAITS for an older transfer. Any scalar DMA
            # emitted before the mm1 loop therefore delays the first silu
            # (and with it the PSUM drain the PE depends on) by tens of
            # us. So scalar gets ONLY the b0-odd/x1-odd front-load here;
            # its remaining transfers (b1/b2-odd, w2-odd) are emitted
            # inside the mm1 loop in batches of <=4 (the slot count) after
            # each quad's swiglu, where the waits are long satisfied.
            w1_t = [[None] * len(W1_BLOCKS) for _ in range(KH)]
            x1_t = []
            for kp in range(KH // 2):
                t = xp.tile([128, 2 * CHUNK], bf16, tag=f"x_1_{kp}", name=f"x1_{kp}")
                x1_t.append(t)
            # Issue order is load-bearing: each queue executes configs in
            # program order, so emission order IS arrival order. sync gets
            # p0e..p2e | x1e | b1e b2e | w2e | x2 x3; scalar gets
            # p0o..p2o | x1o | (batches inside mm1); gpsimd x0 | b3.
            w1_dma = {}  # (b, k) -> (tile, dram slice), deferred issue
            sync_late = []  # b1/b2 k-even, behind x1 on the sync queue
            for b, (p0, npair) in enumerate(W1_BLOCKS):
                c0, cw = p0 * 256, npair * 256
                for k in range(KH):
                    t = w1p.tile([128, cw], bf16, tag=f"w1_{k}_{b}")
                    w1_t[k][b] = t
                    src = w1_d[k * 128 : (k + 1) * 128, c0 : c0 + cw]
                    if b == len(W1_BLOCKS) - 1:
                        nc.gpsimd.dma_start(out=t[:], in_=src)
                    elif b <= 2:
                        eng = nc.sync if k % 2 == 0 else nc.scalar
                        eng.dma_start(out=t[:], in_=src)
                    elif k % 2 == 0:
                        sync_late.append((t, src))
                    else:
                        w1_dma[(b, k)] = (t, src)
            # x chunk 1 rides both HWDGE queues AFTER the first three w1
            # pairs: the stagger below delays its first use to step 2
            # (~34us), so it must not compete with p0-p2 in the front-load.
            # On sync, its pairs are deadline-interleaved with b1-even
            # (x1 kp-th pair is consumed ~1.7us/k from ~36us; b1 from
            # ~43us) so neither stream waits on the other.
            for kp in range(1, KH // 2, 2):
                nc.scalar.dma_start(out=x1_t[kp][:], in_=x_dram_pair(1, kp))
            b1e = sync_late[:8]  # block 3 (pairs 3-5) k-even
            sync_front = [
                ("x", 0), ("x", 2), ("b", 0), ("b", 1),
                ("x", 4), ("b", 2), ("b", 3),
                ("x", 6), ("b", 4), ("b", 5), ("b", 6), ("b", 7),
            ]
            for kind, idx in sync_front:
                if kind == "x":
                    nc.sync.dma_start(out=x1_t[idx][:], in_=x_dram_pair(1, idx))
                else:
                    t, src = b1e[idx]
                    nc.sync.dma_start(out=t[:], in_=src)
            for t, src in sync_late[8:]:
                nc.sync.dma_start(out=t[:], in_=src)
            # Resident w2: 11 tiles [128, 2048]; even k on sync after w1,
            # odd k deferred into the mm1 loop (scalar) — not needed until
            # mm2 of super-chunk 0 (~190us in).
            w2_t = []
            w2_dma = []
            for k in range(NF):
                t = w2p.tile([128, H], bf16, tag=f"w2_{k}")
                w2_t.append(t)
                src = w2_d[k * 128 : (k + 1) * 128, :]
                if k % 2 == 0:
                    nc.sync.dma_start(out=t[:], in_=src)
                else:
                    w2_dma.append((t, src))
            # Scalar-queue batches (<=4 configs each) emitted after step
            # t's swiglu in super-chunk 0: 768-block odds, then w2-odd.
            deferred = [w1_dma[(b, k)] for b in (3, 4) for k in range(1, KH, 2)]
            deferred += w2_dma
            scalar_batches = [deferred[i : i + 4] for i in range(0, len(deferred), 4)]

            for S in range(NSC):
                if S == 0:
                    x_t = [x0_t, x1_t]
                else:
                    # Chunks 2+3 reuse chunk 0+1's SBUF slots (released at
                    # the end of super-chunk 0's mm1); the SP queue is idle
                    # from ~90us so the waits cost nothing.
                    x_t = [[], []]
                    for i in range(NCI):
                        for kp in range(KH // 2):
                            t = xp.tile(
                                [128, 2 * CHUNK], bf16, tag=f"x_{i}_{kp}",
                                name=f"x_{S}_{i}_{kp}",
                            )
                            x_t[i].append(t)
                            nc.sync.dma_start(
                                out=t[:], in_=x_dram_pair(S * NCI + i, kp)
                            )

                # mm1 + swiglu, software-staggered: step t runs pair t of
                # chunk 0 and pair t-2 of chunk 1. Chunk 1's weights are
                # always two pairs old (resident), so only (x0, p0..p2)
                # are on the DMA critical path at startup — x1 isn't
                # needed until step 2 (~34us). Weight demand per block
                # still averages half the v3 rate, which the shared
                # ~290GB/s DMA sustains without PE stalls.
                STAG = 2
                act_t = [[None] * NF, [None] * NF]  # [chunk][j]
                for t_s in range(NF + STAG):
                    chains = []
                    if t_s < NF:
                        chains.append((0, t_s))
                    if t_s >= STAG:
                        chains.append((1, t_s - STAG))
                    for i, j in chains:
                        b = W1_PAIR_BLOCK[j]
                        off = (j - W1_BLOCKS[b][0]) * 256
                        ps_a = psp.tile(
                            [128, CHUNK], f32, tag="ps", name=f"ps_{S}_{i}_{j}_a"
                        )
                        ps_b = psp.tile(
                            [128, CHUNK], f32, tag="ps", name=f"ps_{S}_{i}_{j}_b"
                        )
                        for k in range(KH):
                            w1k = w1_t[k][b]
                            st, sp = (k == 0), (k == KH - 1)
                            xk = x_t[i][k // 2][:, (k % 2) * CHUNK : (k % 2 + 1) * CHUNK]
                            nc.tensor.matmul(
                                ps_a[:], w1k[:, off : off + 128], xk, start=st, stop=sp
                            )
                            nc.tensor.matmul(
                                ps_b[:], w1k[:, off + 128 : off + 256], xk,
                                start=st, stop=sp,
                            )
                        tmp = tmpp.tile([128, CHUNK], f32, tag="tmp")
                        nc.scalar.activation(tmp[:], ps_a[:], SILU)
                        a = actp.tile([128, CHUNK], bf16, tag=f"act_{i}_{j}")
                        act_t[i][j] = a
                        nc.vector.tensor_mul(a[:], tmp[:], ps_b[:])
                    if S == 0 and t_s < len(scalar_batches):
                        for t, src in scalar_batches[t_s]:
                            nc.scalar.dma_start(out=t[:], in_=src)

                # mm2: out[t, h], 8 m-tiles per super-chunk. k-outer/
                # n-inner keeps 4 PSUM banks accumulating; the very last
                # m-tile flips to n-outer so each bank finishes early and
                # its copy + store overlap the remaining matmuls.
                for m in range(NT):
                    i, mc = divmod(m, NT // NCI)
                    last = (S == NSC - 1) and (m == NT - 1)
                    po = [
                        psp.tile([128, 512], f32, tag="ps", name=f"po_{S}_{m}_{n}")
                        for n in range(NHO)
                    ]
                    if last:
                        for n in range(NHO):
                            for k in range(NF):
                                nc.tensor.matmul(
                                    po[n][:],
                                    act_t[i][k][:, mc * 128 : (mc + 1) * 128],
                                    w2_t[k][:, n * 512 : (n + 1) * 512],
                                    start=(k == 0),
                                    stop=(k == NF - 1),
                                )
                    else:
                        for k in range(NF):
                            lhsT = act_t[i][k][:, mc * 128 : (mc + 1) * 128]
                            for n in range(NHO):
                                nc.tensor.matmul(
                                    po[n][:],
                                    lhsT,
                                    w2_t[k][:, n * 512 : (n + 1) * 512],
                                    start=(k == 0),
                                    stop=(k == NF - 1),
                                )
                    r0 = (S * NCI + i) * CHUNK + mc * 128
                    for n in range(NHO):
                        osb = outp.tile([128, 512], bf16, tag="osb")
                        nc.scalar.copy(osb[:], po[n][:])
                        # Final m-tile's store configs ride the (idle) SP
                        # queue: fresh DMA slots + off the scalar stream,
                        # so the kernel tail isn't serialized behind them.
                        seng = nc.sync if last else nc.scalar
                        seng.dma_start(
                            out=out_d[r0 : r0 + 128, n * 512 : (n + 1) * 512],
                            in_=osb[:],
                        )
    if not nc.is_finalized():
        nc.finalize()  # Bacc.finalize runs the lowering pipeline (sem split, alloc_regs)
    return nc


def _get_nc():
    if "nc" not in _CACHE:
        _CACHE["nc"] = _build()
    return _CACHE["nc"]


def _interleave_w1(w1e: np.ndarray) -> np.ndarray:
    """[H, 2816] -> same shape with cols reordered so swiglu pair j
    (a_j = cols [128j,128j+128), b_j = cols [1408+128j, ...)) becomes the
    contiguous range [256j, 256j+256) = [a_j | b_j]."""
    a = w1e[:, :F].reshape(H, NF, 128)
    b = w1e[:, F:].reshape(H, NF, 128)
    return np.stack([a, b], axis=2).reshape(H, F2)


def kernel(permuted_hidden_states, num_tokens_per_expert, w1, w2):
    from concourse.bass_utils import run_bass_kernel_spmd

    x = np.asarray(permuted_hidden_states, dtype=np.float32)
    w1 = np.asarray(w1, dtype=np.float32)
    w2 = np.asarray(w2, dtype=np.float32)
    ntpe = np.asarray(num_tokens_per_expert)
    assert x.shape == (T_TOTAL, H) and w1.shape == (E, H, F2) and w2.shape == (E, F, H)
    # Reference semantics rely on the static equal split.
    assert np.all(ntpe == TPC), f"expected equal {TPC}-token splits, got {ntpe}"

    bf = ml_dtypes.bfloat16
    in_maps = []
    NCH = TPC // CHUNK
    for e in range(E):
        xe = x[e * TPC : (e + 1) * TPC]
        # pack as [p, chunk, k, t] so any k-range of a chunk is one
        # contiguous 2D DMA slice (see xT_d comment in _build).
        xg = (
            xe.reshape(NCH, CHUNK, KH, 128)
            .transpose(3, 0, 2, 1)
            .reshape(128, NCH * KH * CHUNK)
        )
        in_maps.append(
            {
                "xT": np.ascontiguousarray(xg).astype(bf),
                "w1": np.ascontiguousarray(_interleave_w1(w1[e])).astype(bf),
                "w2": np.ascontiguousarray(w2[e]).astype(bf),
            }
        )

    nc = _get_nc()
    res = run_bass_kernel_spmd(nc, in_maps, list(range(E)), trace=TRACE)
    LAST["exec_time_ns"] = res.exec_time_ns
    LAST["mean_exec_time_ns"] = res.mean_exec_time_ns
    LAST["profile_json"] = res.profile_json
    out = np.concatenate([res.results[i]["out"] for i in range(E)], axis=0)
    return np.ascontiguousarray(out.astype(np.float32))
